# revision 8
# baseline (speedup 1.0000x reference)
"""Trainium2 Bass kernel for the bug-faithful CTRNN problem.

Semantics (hidden is never updated across time in the reference):
    out[t] = relu(x[t] @ W_ih^T + b_ih + hidden @ W_hh^T + b_hh)
    return out, out[-1]

This collapses to one large matmul over (T*B, I) @ (I, H) plus a per-batch
bias term shared across time.

Distribution: data-parallel over T across 8 NeuronCores (each core gets
T/8 = 32 timesteps = 4096 rows); W_ih / W_hh / hidden / biases replicated.

Device mapping per core:
  - lhsT (stationary) = x^T tiles [i=128, r=128], fp32r (fp22-truncated fp32,
    full PE rate at N>=256)
  - rhs  (moving)     = W_ih^T tiles [i=128, h=512], fp32r
  - psum [r=128, h=512] accumulated over 4 k-tiles
  - bias_full[b, h] = hidden @ W_hh^T + b_ih + b_hh computed once on device
    via an augmented contraction (two ones-rows pick up b_ih and b_hh)
  - epilogue: DVE add (psum + bias) -> fp16 SBUF, ACT relu, DMA to DRAM
  - host: upcast fp16 -> fp32, concat shards
"""

import sys

if "/opt/trn_rl_repo" not in sys.path:
    sys.path.insert(0, "/opt/trn_rl_repo")

import numpy as np

import concourse.bacc as bacc
import concourse.bass as bass  # noqa: F401
import concourse.mybir as mybir
from concourse.bass_utils import run_bass_kernel_spmd
from concourse.tile import TileContext

T, B, I, H = 256, 128, 512, 1024
NCORES = 8
TCORE = T // NCORES          # timesteps per core
R = TCORE * B                # rows per core (4096)
KI = I // 128                # k-tiles for the input matmul (4)
KH = H // 128 + 1            # k-tiles for the hidden matmul, incl. bias rows (9)
RCHUNK = 512                 # rows per x DMA chunk
NCHUNKS = R // RCHUNK        # 8
RTILES = RCHUNK // 128       # r-tiles per chunk (4)

OUT_DT = mybir.dt.float16    # device output dtype (upcast to fp32 on host)

LAST_EXEC_NS = None          # filled in by run() when tracing is enabled
LAST_RESULT = None

_NC_CACHE = {}


def _build(reps=1):
    f32r = mybir.dt.float32r
    f32 = mybir.dt.float32
    relu = mybir.ActivationFunctionType.Relu

    nc = bacc.Bacc("TRN2", target_bir_lowering=False)
    xt_d = nc.declare_dram_parameter(
        "xt", [NCHUNKS, 128, KI, RCHUNK], f32r, isOutput=False
    )
    wt_d = nc.declare_dram_parameter("wt", [128, KI, H], f32r, isOutput=False)
    hid_d = nc.declare_dram_parameter("hid", [128, KH, B], f32r, isOutput=False)
    whh_d = nc.declare_dram_parameter("whh", [128, KH, H], f32r, isOutput=False)
    out_d = nc.declare_dram_parameter("out", [R // 128, 128, H], OUT_DT, isOutput=True)

    with TileContext(nc) as tc:
        with (
            tc.tile_pool(name="wpool", bufs=1) as wpool,
            tc.tile_pool(name="hpool", bufs=1) as hpool,
            tc.tile_pool(name="xpool", bufs=4) as xpool,
            tc.tile_pool(name="opool", bufs=6) as opool,
            tc.tile_pool(name="psum_main", bufs=6, space="PSUM") as psum_main,
            tc.tile_pool(name="psum_bias", bufs=2, space="PSUM") as psum_bias,
        ):
            # Input loads on the SP (sync) HWDGE ring.
            wt_sb = wpool.tile([128, KI, H], f32r)
            nc.sync.dma_start(out=wt_sb[:], in_=wt_d[:])

            x_tiles = []
            xc0 = xpool.tile([128, KI, RCHUNK], f32r)
            nc.sync.dma_start(out=xc0[:], in_=xt_d[:][0])
            x_tiles.append(xc0)

            # Hidden-term operands on the ACT (scalar) HWDGE ring, which is
            # otherwise idle early (it carries the output stores later).
            hid_sb = hpool.tile([128, KH, B], f32r)
            nc.scalar.dma_start(out=hid_sb[:], in_=hid_d[:])
            whh_sb = hpool.tile([128, KH, H], f32r)
            nc.scalar.dma_start(out=whh_sb[:], in_=whh_d[:])

            bias_sb = wpool.tile([128, H], f32)

            def emit_bias_stage():
                for h2 in range(2):
                    ps = psum_bias.tile([128, 512], f32)
                    for kk in range(KH):
                        nc.tensor.matmul(
                            ps[:],
                            lhsT=hid_sb[:, kk, :],
                            rhs=whh_sb[:, kk, h2 * 512 : (h2 + 1) * 512],
                            start=(kk == 0),
                            stop=(kk == KH - 1),
                        )
                    nc.vector.tensor_copy(
                        out=bias_sb[:, h2 * 512 : (h2 + 1) * 512], in_=ps[:]
                    )

            def emit_rtile_mms(xc, rt):
                ps_pair = []
                for h2 in range(2):
                    ps = psum_main.tile([128, 512], f32)
                    for kk in range(KI):
                        nc.tensor.matmul(
                            ps[:],
                            lhsT=xc[:, kk, rt * 128 : (rt + 1) * 128],
                            rhs=wt_sb[:, kk, h2 * 512 : (h2 + 1) * 512],
                            start=(kk == 0),
                            stop=(kk == KI - 1),
                        )
                    ps_pair.append(ps)
                return ps_pair

            def emit_rtile_epilogue(ps_pair, m):
                o_sb = opool.tile([128, H], OUT_DT)
                for h2 in range(2):
                    nc.vector.tensor_add(
                        out=o_sb[:, h2 * 512 : (h2 + 1) * 512],
                        in0=ps_pair[h2][:],
                        in1=bias_sb[:, h2 * 512 : (h2 + 1) * 512],
                    )
                nc.scalar.activation(out=o_sb[:], in_=o_sb[:], func=relu)
                nc.scalar.dma_start(out=out_d[:][m], in_=o_sb[:])

            def emit_rtile(xc, m, rt):
                emit_rtile_epilogue(emit_rtile_mms(xc, rt), m)

            # Chunk 0, r-tile 0 matmuls first so PE starts as soon as wt+xc0
            # land; then the bias stage (its DVE copies must precede r-tile
            # 0's bias-consuming adds in the DVE stream, and its matmuls must
            # precede any matmul that transitively waits on bias consumers ->
            # no psum-slot deadlock); then r-tile 0's epilogue.
            ps0 = emit_rtile_mms(xc0, 0)
            emit_bias_stage()
            emit_rtile_epilogue(ps0, 0)
            for rt in range(1, RTILES):
                emit_rtile(xc0, rt, rt)
            for c in range(1, NCHUNKS):
                xc = xpool.tile([128, KI, RCHUNK], f32r)
                nc.sync.dma_start(out=xc[:], in_=xt_d[:][c])
                for rt in range(RTILES):
                    emit_rtile(xc, c * RTILES + rt, rt)

            # Extra repetitions of the full body (timing runs only; the
            # output is just rewritten with identical values).
            for _rep in range(1, reps):
                for c in range(NCHUNKS):
                    xc = xpool.tile([128, KI, RCHUNK], f32r)
                    nc.sync.dma_start(out=xc[:], in_=xt_d[:][c])
                    for rt in range(RTILES):
                        emit_rtile(xc, c * RTILES + rt, rt)

    nc.compile()  # bacc pass pipeline: legalizes multi-wait instructions etc.
    return nc


def _get_nc(reps=1):
    if reps not in _NC_CACHE:
        _NC_CACHE[reps] = _build(reps)
    return _NC_CACHE[reps]


def _prep_inputs(x, hidden, W_ih, W_hh, b_ih, b_hh):
    x = np.ascontiguousarray(np.asarray(x, dtype=np.float32))
    hidden = np.asarray(hidden, dtype=np.float32)
    W_ih = np.asarray(W_ih, dtype=np.float32)
    W_hh = np.asarray(W_hh, dtype=np.float32)
    b_ih = np.asarray(b_ih, dtype=np.float32)
    b_hh = np.asarray(b_hh, dtype=np.float32)

    # wt[p, k, h] = W_ih[h, k*128+p]
    wt = np.ascontiguousarray(W_ih.T.reshape(KI, 128, H).transpose(1, 0, 2))

    # Augmented hidden^T: rows 0..H-1 = hidden.T, rows H and H+1 = 1.0 (they
    # pick up the b_ih and b_hh rows of the augmented W_hh^T), rest zero.
    ha = np.zeros((KH * 128, B), dtype=np.float32)
    ha[:H] = hidden.T
    ha[H] = 1.0
    ha[H + 1] = 1.0
    hid = np.ascontiguousarray(ha.reshape(KH, 128, B).transpose(1, 0, 2))

    wa = np.zeros((KH * 128, H), dtype=np.float32)
    wa[:H] = W_hh.T
    wa[H] = b_ih
    wa[H + 1] = b_hh
    whh = np.ascontiguousarray(wa.reshape(KH, 128, H).transpose(1, 0, 2))

    in_maps = []
    for c in range(NCORES):
        xs = x[c * TCORE : (c + 1) * TCORE].reshape(R, I)
        # xt[c8, p, k, r] = xs[c8*RCHUNK + r, k*128 + p]
        xt = np.ascontiguousarray(
            xs.reshape(NCHUNKS, RCHUNK, KI, 128).transpose(0, 3, 2, 1)
        )
        in_maps.append({"xt": xt, "wt": wt, "hid": hid, "whh": whh})
    return in_maps


def run(inputs, trace=False, tmpdir=None, reps=1):
    global LAST_EXEC_NS, LAST_RESULT
    nc = _get_nc(reps)
    in_maps = _prep_inputs(**inputs)
    res = run_bass_kernel_spmd(
        nc, in_maps, core_ids=list(range(NCORES)), trace=trace, tmpdir=tmpdir
    )
    LAST_RESULT = res
    LAST_EXEC_NS = res.exec_time_ns
    out = np.empty((T, B, H), dtype=np.float32)
    for c in range(NCORES):
        shard = res.results[c]["out"]  # (R//128, 128, H) fp16
        out[c * TCORE : (c + 1) * TCORE] = shard.astype(np.float32).reshape(
            TCORE, B, H
        )
    return out, out[-1]


def kernel(x, hidden, W_ih, W_hh, b_ih, b_hh):
    return run(
        dict(x=x, hidden=hidden, W_ih=W_ih, W_hh=W_hh, b_ih=b_ih, b_hh=b_hh)
    )


# revision 12
# speedup vs baseline: 286.6415x; 286.6415x over previous
"""Trainium2 Bass kernel for the bug-faithful CTRNN problem.

Semantics (hidden is never updated across time in the reference):
    out[t] = relu(x[t] @ W_ih^T + b_ih + hidden @ W_hh^T + b_hh)
    return out, out[-1]

This collapses to one large matmul over (T*B, I) @ (I, H) plus a per-batch
bias term shared across time.

Distribution: data-parallel over T across 8 NeuronCores (each core gets
T/8 = 32 timesteps = 4096 rows); W_ih / W_hh / hidden / biases replicated.

Device mapping per core:
  - lhsT (stationary) = x^T tiles [i=128, r=128], fp32r (fp22-truncated fp32,
    full PE rate at N>=256)
  - rhs  (moving)     = W_ih^T tiles [i=128, h=512], fp32r
  - psum [r=128, h=512] accumulated over 4 k-tiles
  - bias_full[b, h] = hidden @ W_hh^T + b_ih + b_hh computed once on device
    via an augmented contraction (two ones-rows pick up b_ih and b_hh)
  - epilogue: DVE add (psum + bias) -> fp16 SBUF, ACT relu, DMA to DRAM
  - host: upcast fp16 -> fp32, concat shards
"""

import sys

if "/opt/trn_rl_repo" not in sys.path:
    sys.path.insert(0, "/opt/trn_rl_repo")

import numpy as np

import concourse.bacc as bacc
import concourse.bass as bass  # noqa: F401
import concourse.mybir as mybir
from concourse.bass_utils import run_bass_kernel_spmd
from concourse.tile import TileContext

T, B, I, H = 256, 128, 512, 1024
NCORES = 8
TCORE = T // NCORES          # timesteps per core
R = TCORE * B                # rows per core (4096)
KI = I // 128                # k-tiles for the input matmul (4)
KH = H // 128 + 1            # k-tiles for the hidden matmul, incl. bias rows (9)
RCHUNK = 512                 # rows per x DMA chunk
NCHUNKS = R // RCHUNK        # 8
RTILES = RCHUNK // 128       # r-tiles per chunk (4)

OUT_DT = mybir.dt.float16    # device output dtype (upcast to fp32 on host)

LAST_EXEC_NS = None          # filled in by run() when tracing is enabled
LAST_RESULT = None

_NC_CACHE = {}


def _build(reps=1, loop_n=None):
    f32r = mybir.dt.float32r
    f32 = mybir.dt.float32
    relu = mybir.ActivationFunctionType.Relu

    nc = bacc.Bacc("TRN2", target_bir_lowering=False)
    xt_d = nc.declare_dram_parameter(
        "xt", [NCHUNKS, 128, KI, RCHUNK], f32r, isOutput=False
    )
    wt_d = nc.declare_dram_parameter("wt", [128, KI, H], f32r, isOutput=False)
    hid_d = nc.declare_dram_parameter("hid", [128, KH, B], f32r, isOutput=False)
    whh_d = nc.declare_dram_parameter("whh", [128, KH, H], f32r, isOutput=False)
    out_d = nc.declare_dram_parameter("out", [R // 128, 128, H], OUT_DT, isOutput=True)

    with TileContext(nc) as tc:
        with (
            tc.tile_pool(name="wpool", bufs=2) as wpool,
            tc.tile_pool(name="hpool", bufs=2) as hpool,
            tc.tile_pool(name="xpool", bufs=4) as xpool,
            tc.tile_pool(name="opool", bufs=6) as opool,
            tc.tile_pool(name="psum_main", bufs=6, space="PSUM") as psum_main,
            tc.tile_pool(name="psum_bias", bufs=2, space="PSUM") as psum_bias,
        ):

            def emit_body():
                # Input loads on the SP (sync) HWDGE ring.
                wt_sb = wpool.tile([128, KI, H], f32r)
                nc.sync.dma_start(out=wt_sb[:], in_=wt_d[:])

                xc0 = xpool.tile([128, KI, RCHUNK], f32r)
                nc.sync.dma_start(out=xc0[:], in_=xt_d[:][0])

                # Hidden-term operands on the ACT (scalar) HWDGE ring, which
                # is otherwise idle early (it carries output stores later).
                hid_sb = hpool.tile([128, KH, B], f32r)
                nc.scalar.dma_start(out=hid_sb[:], in_=hid_d[:])
                whh_sb = hpool.tile([128, KH, H], f32r)
                nc.scalar.dma_start(out=whh_sb[:], in_=whh_d[:])

                bias_sb = wpool.tile([128, H], f32)

                def emit_bias_stage():
                    for h2 in range(2):
                        ps = psum_bias.tile([128, 512], f32)
                        for kk in range(KH):
                            nc.tensor.matmul(
                                ps[:],
                                lhsT=hid_sb[:, kk, :],
                                rhs=whh_sb[:, kk, h2 * 512 : (h2 + 1) * 512],
                                start=(kk == 0),
                                stop=(kk == KH - 1),
                            )
                        nc.vector.tensor_copy(
                            out=bias_sb[:, h2 * 512 : (h2 + 1) * 512], in_=ps[:]
                        )

                def emit_rtile_mms(xc, rt):
                    ps_pair = []
                    for h2 in range(2):
                        ps = psum_main.tile([128, 512], f32)
                        for kk in range(KI):
                            nc.tensor.matmul(
                                ps[:],
                                lhsT=xc[:, kk, rt * 128 : (rt + 1) * 128],
                                rhs=wt_sb[:, kk, h2 * 512 : (h2 + 1) * 512],
                                start=(kk == 0),
                                stop=(kk == KI - 1),
                            )
                        ps_pair.append(ps)
                    return ps_pair

                def emit_rtile_epilogue(ps_pair, m):
                    o_sb = opool.tile([128, H], OUT_DT)
                    for h2 in range(2):
                        nc.vector.tensor_add(
                            out=o_sb[:, h2 * 512 : (h2 + 1) * 512],
                            in0=ps_pair[h2][:],
                            in1=bias_sb[:, h2 * 512 : (h2 + 1) * 512],
                        )
                    nc.scalar.activation(out=o_sb[:], in_=o_sb[:], func=relu)
                    nc.scalar.dma_start(out=out_d[:][m], in_=o_sb[:])

                def emit_rtile(xc, m, rt):
                    emit_rtile_epilogue(emit_rtile_mms(xc, rt), m)

                # Chunk 0, r-tile 0 matmuls first so PE starts as soon as
                # wt+xc0 land; then the bias stage (its DVE copies must
                # precede r-tile 0's bias-consuming adds in the DVE stream,
                # and its matmuls must precede any matmul that transitively
                # waits on bias consumers -> no psum-slot deadlock); then
                # r-tile 0's epilogue.
                ps0 = emit_rtile_mms(xc0, 0)
                emit_bias_stage()
                emit_rtile_epilogue(ps0, 0)
                for rt in range(1, RTILES):
                    emit_rtile(xc0, rt, rt)
                for c in range(1, NCHUNKS):
                    xc = xpool.tile([128, KI, RCHUNK], f32r)
                    nc.sync.dma_start(out=xc[:], in_=xt_d[:][c])
                    for rt in range(RTILES):
                        emit_rtile(xc, c * RTILES + rt, rt)

            if loop_n is not None:
                # Timing-only variant: repeat the whole body in a hardware
                # loop so device exec time dominates host/transfer overhead.
                with tc.For_i(0, loop_n, 1, hint_engines=(mybir.EngineType.PE,)):
                    emit_body()
            else:
                for _ in range(reps):
                    emit_body()

    nc.compile()  # bacc pass pipeline: legalizes multi-wait instructions etc.
    return nc


def _get_nc(reps=1, loop_n=None):
    key = (reps, loop_n)
    if key not in _NC_CACHE:
        _NC_CACHE[key] = _build(reps, loop_n)
    return _NC_CACHE[key]


def _prep_inputs(x, hidden, W_ih, W_hh, b_ih, b_hh):
    x = np.ascontiguousarray(np.asarray(x, dtype=np.float32))
    hidden = np.asarray(hidden, dtype=np.float32)
    W_ih = np.asarray(W_ih, dtype=np.float32)
    W_hh = np.asarray(W_hh, dtype=np.float32)
    b_ih = np.asarray(b_ih, dtype=np.float32)
    b_hh = np.asarray(b_hh, dtype=np.float32)

    # wt[p, k, h] = W_ih[h, k*128+p]
    wt = np.ascontiguousarray(W_ih.T.reshape(KI, 128, H).transpose(1, 0, 2))

    # Augmented hidden^T: rows 0..H-1 = hidden.T, rows H and H+1 = 1.0 (they
    # pick up the b_ih and b_hh rows of the augmented W_hh^T), rest zero.
    ha = np.zeros((KH * 128, B), dtype=np.float32)
    ha[:H] = hidden.T
    ha[H] = 1.0
    ha[H + 1] = 1.0
    hid = np.ascontiguousarray(ha.reshape(KH, 128, B).transpose(1, 0, 2))

    wa = np.zeros((KH * 128, H), dtype=np.float32)
    wa[:H] = W_hh.T
    wa[H] = b_ih
    wa[H + 1] = b_hh
    whh = np.ascontiguousarray(wa.reshape(KH, 128, H).transpose(1, 0, 2))

    in_maps = []
    for c in range(NCORES):
        xs = x[c * TCORE : (c + 1) * TCORE].reshape(R, I)
        # xt[c8, p, k, r] = xs[c8*RCHUNK + r, k*128 + p]
        xt = np.ascontiguousarray(
            xs.reshape(NCHUNKS, RCHUNK, KI, 128).transpose(0, 3, 2, 1)
        )
        in_maps.append({"xt": xt, "wt": wt, "hid": hid, "whh": whh})
    return in_maps


def run(inputs, trace=False, tmpdir=None, reps=1, loop_n=None):
    global LAST_EXEC_NS, LAST_RESULT
    nc = _get_nc(reps, loop_n)
    in_maps = _prep_inputs(**inputs)
    res = run_bass_kernel_spmd(
        nc, in_maps, core_ids=list(range(NCORES)), trace=trace, tmpdir=tmpdir
    )
    LAST_RESULT = res
    LAST_EXEC_NS = res.exec_time_ns
    out = np.empty((T, B, H), dtype=np.float32)
    for c in range(NCORES):
        shard = res.results[c]["out"]  # (R//128, 128, H) fp16
        out[c * TCORE : (c + 1) * TCORE] = shard.astype(np.float32).reshape(
            TCORE, B, H
        )
    return out, out[-1]


def kernel(x, hidden, W_ih, W_hh, b_ih, b_hh):
    return run(
        dict(x=x, hidden=hidden, W_ih=W_ih, W_hh=W_hh, b_ih=b_ih, b_hh=b_hh)
    )


# revision 24
# speedup vs baseline: 356.3337x; 1.2431x over previous
"""Trainium2 Bass kernel for the bug-faithful CTRNN problem.

Semantics (hidden is never updated across time in the reference):
    out[t] = relu(x[t] @ W_ih^T + b_ih + hidden @ W_hh^T + b_hh)
    return out, out[-1]

This collapses to one large matmul over (T*B, I) @ (I, H) plus a per-batch
bias term shared across time.

Distribution: data-parallel over T across 8 NeuronCores (each core gets
T/8 = 32 timesteps = 4096 rows); W_ih / W_hh / hidden / biases replicated.

Device mapping per core:
  - lhsT (stationary) = x^T tiles [i=128, r=128], fp32r (fp22-truncated fp32,
    full PE rate at N>=256)
  - rhs  (moving)     = W_ih^T tiles [i=128, h=512], fp32r
  - psum [r=128, h=512] accumulated over 4 k-tiles
  - bias_full[b, h] = hidden @ W_hh^T + b_ih + b_hh computed once on device
    via an augmented contraction (two ones-rows pick up b_ih and b_hh)
  - epilogue: DVE add (psum + bias) -> fp16 SBUF, ACT relu, DMA to DRAM
  - host: upcast fp16 -> fp32, concat shards
"""

import sys

if "/opt/trn_rl_repo" not in sys.path:
    sys.path.insert(0, "/opt/trn_rl_repo")

import numpy as np

import concourse.bacc as bacc
import concourse.bass as bass  # noqa: F401
import concourse.mybir as mybir
from concourse.bass_utils import run_bass_kernel_spmd
from concourse.tile import TileContext

T, B, I, H = 256, 128, 512, 1024
NCORES = 8
TCORE = T // NCORES          # timesteps per core
R = TCORE * B                # rows per core (4096)
KI = I // 128                # k-tiles for the input matmul (4)
KH = H // 128 + 1            # k-tiles for the hidden matmul, incl. bias rows (9)
RCHUNK = 512                 # rows per x DMA chunk
NCHUNKS = R // RCHUNK        # 8
RTILES = RCHUNK // 128       # r-tiles per chunk (4)

OUT_DT = mybir.dt.float16    # device output dtype (upcast to fp32 on host)

LAST_EXEC_NS = None          # filled in by run() when tracing is enabled
LAST_RESULT = None

_NC_CACHE = {}


def _build(reps=1, loop_n=None, variant="full"):
    f32r = mybir.dt.float32r
    f32 = mybir.dt.float32
    f16 = mybir.dt.float16
    relu = mybir.ActivationFunctionType.Relu

    nc = bacc.Bacc("TRN2", target_bir_lowering=False)
    xt_d = nc.declare_dram_parameter(
        "xt", [NCHUNKS, 128, KI, RCHUNK], f16, isOutput=False
    )
    wt_d = nc.declare_dram_parameter("wt", [128, KI, H], f16, isOutput=False)
    hid_d = nc.declare_dram_parameter("hid", [128, KH, B], f16, isOutput=False)
    whh_d = nc.declare_dram_parameter("whh", [128, KH, H], f16, isOutput=False)
    out_d = nc.declare_dram_parameter("out", [R // 128, 128, H], OUT_DT, isOutput=True)

    with TileContext(nc) as tc:
        wh_bufs = 2 if variant == "full" else 1
        with (
            tc.tile_pool(name="wpool", bufs=wh_bufs) as wpool,
            tc.tile_pool(name="hpool", bufs=wh_bufs) as hpool,
            tc.tile_pool(name="xpool", bufs=4) as xpool,
            tc.tile_pool(name="opool", bufs=6) as opool,
            tc.tile_pool(name="psum_main", bufs=6, space="PSUM") as psum_main,
            tc.tile_pool(name="psum_bias", bufs=2, space="PSUM") as psum_bias,
        ):

            def emit_body():
                # Input loads on the SP (sync) HWDGE ring.
                wt_sb = wpool.tile([128, KI, H], f16)
                nc.sync.dma_start(out=wt_sb[:], in_=wt_d[:])

                xc0 = xpool.tile([128, KI, RCHUNK], f16)
                nc.sync.dma_start(out=xc0[:], in_=xt_d[:][0])

                # Hidden-term operands on the ACT (scalar) HWDGE ring, which
                # is otherwise idle early (it carries output stores later).
                hid_sb = hpool.tile([128, KH, B], f16)
                nc.scalar.dma_start(out=hid_sb[:], in_=hid_d[:])
                whh_sb = hpool.tile([128, KH, H], f16)
                nc.scalar.dma_start(out=whh_sb[:], in_=whh_d[:])

                bias_sb = wpool.tile([128, H], f32)

                def emit_bias_stage():
                    for h2 in range(2):
                        ps = psum_bias.tile([128, 512], f32)
                        for kk in range(KH):
                            nc.tensor.matmul(
                                ps[:],
                                lhsT=hid_sb[:, kk, :],
                                rhs=whh_sb[:, kk, h2 * 512 : (h2 + 1) * 512],
                                start=(kk == 0),
                                stop=(kk == KH - 1),
                            )
                        nc.vector.tensor_copy(
                            out=bias_sb[:, h2 * 512 : (h2 + 1) * 512], in_=ps[:]
                        )

                def emit_rtile_mms(xc, rt):
                    ps_pair = []
                    for h2 in range(2):
                        ps = psum_main.tile([128, 512], f32)
                        for kk in range(KI):
                            nc.tensor.matmul(
                                ps[:],
                                lhsT=xc[:, kk, rt * 128 : (rt + 1) * 128],
                                rhs=wt_sb[:, kk, h2 * 512 : (h2 + 1) * 512],
                                start=(kk == 0),
                                stop=(kk == KI - 1),
                            )
                        ps_pair.append(ps)
                    return ps_pair

                def emit_rtile_epilogue(ps_pair, m):
                    o_sb = opool.tile([128, H], OUT_DT)
                    for h2 in range(2):
                        nc.vector.tensor_add(
                            out=o_sb[:, h2 * 512 : (h2 + 1) * 512],
                            in0=ps_pair[h2][:],
                            in1=bias_sb[:, h2 * 512 : (h2 + 1) * 512],
                        )
                    nc.scalar.activation(out=o_sb[:], in_=o_sb[:], func=relu)
                    nc.scalar.dma_start(out=out_d[:][m], in_=o_sb[:])

                def emit_rtile(xc, m, rt):
                    emit_rtile_epilogue(emit_rtile_mms(xc, rt), m)

                # Chunk 0, r-tile 0 matmuls first so PE starts as soon as
                # wt+xc0 land; then the bias stage (its DVE copies must
                # precede r-tile 0's bias-consuming adds in the DVE stream,
                # and its matmuls must precede any matmul that transitively
                # waits on bias consumers -> no psum-slot deadlock); then
                # r-tile 0's epilogue.
                ps0 = emit_rtile_mms(xc0, 0)
                emit_bias_stage()
                emit_rtile_epilogue(ps0, 0)
                for rt in range(1, RTILES):
                    emit_rtile(xc0, rt, rt)
                for c in range(1, NCHUNKS):
                    xc = xpool.tile([128, KI, RCHUNK], f16)
                    nc.sync.dma_start(out=xc[:], in_=xt_d[:][c])
                    for rt in range(RTILES):
                        emit_rtile(xc, c * RTILES + rt, rt)

            def emit_variant_body(kind, x_tiles, wt_sb, hid_sb, whh_sb, o_sb):
                if kind == "mm":
                    for c in range(NCHUNKS):
                        for rt in range(RTILES):
                            for h2 in range(2):
                                ps = psum_main.tile([128, 512], f32)
                                for kk in range(KI):
                                    nc.tensor.matmul(
                                        ps[:],
                                        lhsT=x_tiles[c][:, kk, rt * 128 : (rt + 1) * 128],
                                        rhs=wt_sb[:, kk, h2 * 512 : (h2 + 1) * 512],
                                        start=(kk == 0),
                                        stop=(kk == KI - 1),
                                    )
                elif kind == "mm_shared":
                    for c in range(NCHUNKS):
                        for rt in range(RTILES):
                            for h2 in range(2):
                                ps = psum_main.tile([128, 512], f32)
                                for kk in range(KI):
                                    nc.tensor.matmul(
                                        ps[:],
                                        lhsT=x_tiles[0][:, 0, 0:128],
                                        rhs=wt_sb[:, 0, 0:512],
                                        start=(kk == 0),
                                        stop=(kk == KI - 1),
                                    )
                elif kind == "mm_sharedw":
                    # fixed stationary, varying moving operand
                    for c in range(NCHUNKS):
                        for rt in range(RTILES):
                            for h2 in range(2):
                                ps = psum_main.tile([128, 512], f32)
                                for kk in range(KI):
                                    nc.tensor.matmul(
                                        ps[:],
                                        lhsT=x_tiles[0][:, 0, 0:128],
                                        rhs=wt_sb[:, kk, h2 * 512 : (h2 + 1) * 512],
                                        start=(kk == 0),
                                        stop=(kk == KI - 1),
                                    )
                elif kind == "mm_reuse8":
                    # stationary = Wt tile reused across 8 consecutive MMs
                    # (transposed-output mapping)
                    for ht in range(H // 128):
                        pss = [
                            psum_main.tile(
                                [128, 512], f32, bufs=8, tag="ps8", name=f"ps8_{ht}_{i}"
                            )
                            for i in range(NCHUNKS)
                        ]
                        for kk in range(KI):
                            for rc in range(NCHUNKS):
                                nc.tensor.matmul(
                                    pss[rc][:],
                                    lhsT=wt_sb[:, kk, ht * 128 : (ht + 1) * 128],
                                    rhs=x_tiles[rc][:, kk, :],
                                    start=(kk == 0),
                                    stop=(kk == KI - 1),
                                )
                elif kind == "dma":
                    wt2 = wpool.tile([128, KI, H], f16, bufs=1, tag="wt2")
                    nc.sync.dma_start(out=wt2[:], in_=wt_d[:])
                    hid2 = hpool.tile([128, KH, B], f16, bufs=1, tag="hid2")
                    nc.scalar.dma_start(out=hid2[:], in_=hid_d[:])
                    whh2 = hpool.tile([128, KH, H], f16, bufs=1, tag="whh2")
                    nc.scalar.dma_start(out=whh2[:], in_=whh_d[:])
                    for c in range(NCHUNKS):
                        xc = xpool.tile([128, KI, RCHUNK], f16)
                        nc.sync.dma_start(out=xc[:], in_=xt_d[:][c])
                        for rt in range(RTILES):
                            nc.scalar.dma_start(
                                out=out_d[:][c * RTILES + rt], in_=o_sb[:]
                            )

            if variant != "full":
                # Bottleneck-bisection variants (timing only, wrong output).
                wt_sb = wpool.tile([128, KI, H], f16)
                nc.sync.dma_start(out=wt_sb[:], in_=wt_d[:])
                hid_sb = hpool.tile([128, KH, B], f16)
                nc.scalar.dma_start(out=hid_sb[:], in_=hid_d[:])
                whh_sb = hpool.tile([128, KH, H], f16)
                nc.scalar.dma_start(out=whh_sb[:], in_=whh_d[:])
                o_sb = opool.tile([128, H], OUT_DT)
                nc.any.memset(o_sb[:], 0.0)
                x_tiles = []
                if variant != "dma":
                    for c in range(NCHUNKS):
                        xc = xpool.tile(
                            [128, KI, RCHUNK], f16, bufs=NCHUNKS, tag="xres"
                        )
                        nc.sync.dma_start(out=xc[:], in_=xt_d[:][c])
                        x_tiles.append(xc)
                assert loop_n is not None
                with tc.For_i(0, loop_n, 1, hint_engines=(mybir.EngineType.PE,)):
                    emit_variant_body(variant, x_tiles, wt_sb, hid_sb, whh_sb, o_sb)
            elif loop_n is not None:
                # Timing-only variant: repeat the whole body in a hardware
                # loop so device exec time dominates host/transfer overhead.
                with tc.For_i(0, loop_n, 1, hint_engines=(mybir.EngineType.PE,)):
                    emit_body()
            else:
                for _ in range(reps):
                    emit_body()

    nc.compile()  # bacc pass pipeline: legalizes multi-wait instructions etc.
    return nc


def _get_nc(reps=1, loop_n=None, variant="full"):
    key = (reps, loop_n, variant)
    if key not in _NC_CACHE:
        _NC_CACHE[key] = _build(reps, loop_n, variant)
    return _NC_CACHE[key]


def _prep_inputs(x, hidden, W_ih, W_hh, b_ih, b_hh):
    x = np.ascontiguousarray(np.asarray(x, dtype=np.float32))
    hidden = np.asarray(hidden, dtype=np.float32)
    W_ih = np.asarray(W_ih, dtype=np.float32)
    W_hh = np.asarray(W_hh, dtype=np.float32)
    b_ih = np.asarray(b_ih, dtype=np.float32)
    b_hh = np.asarray(b_hh, dtype=np.float32)

    # wt[p, k, h] = W_ih[h, k*128+p], fp16 moving operand
    wt = np.ascontiguousarray(
        W_ih.T.reshape(KI, 128, H).transpose(1, 0, 2).astype(np.float16)
    )

    # Augmented hidden^T: rows 0..H-1 = hidden.T, rows H and H+1 = 1.0 (they
    # pick up the b_ih and b_hh rows of the augmented W_hh^T), rest zero.
    ha = np.zeros((KH * 128, B), dtype=np.float32)
    ha[:H] = hidden.T
    ha[H] = 1.0
    ha[H + 1] = 1.0
    hid = np.ascontiguousarray(
        ha.reshape(KH, 128, B).transpose(1, 0, 2).astype(np.float16)
    )

    wa = np.zeros((KH * 128, H), dtype=np.float32)
    wa[:H] = W_hh.T
    wa[H] = b_ih
    wa[H + 1] = b_hh
    whh = np.ascontiguousarray(
        wa.reshape(KH, 128, H).transpose(1, 0, 2).astype(np.float16)
    )

    in_maps = []
    for c in range(NCORES):
        xs = x[c * TCORE : (c + 1) * TCORE].reshape(R, I)
        # xt[c8, p, k, r] = xs[c8*RCHUNK + r, k*128 + p]
        xt = np.ascontiguousarray(
            xs.reshape(NCHUNKS, RCHUNK, KI, 128).transpose(0, 3, 2, 1)
        ).astype(np.float16)
        in_maps.append({"xt": xt, "wt": wt, "hid": hid, "whh": whh})
    return in_maps


def run(inputs, trace=False, tmpdir=None, reps=1, loop_n=None, variant="full"):
    global LAST_EXEC_NS, LAST_RESULT
    nc = _get_nc(reps, loop_n, variant)
    in_maps = _prep_inputs(**inputs)
    res = run_bass_kernel_spmd(
        nc, in_maps, core_ids=list(range(NCORES)), trace=trace, tmpdir=tmpdir
    )
    LAST_RESULT = res
    LAST_EXEC_NS = res.exec_time_ns
    out = np.empty((T, B, H), dtype=np.float32)
    for c in range(NCORES):
        shard = res.results[c]["out"]  # (R//128, 128, H) fp16
        out[c * TCORE : (c + 1) * TCORE] = shard.astype(np.float32).reshape(
            TCORE, B, H
        )
    return out, out[-1]


def kernel(x, hidden, W_ih, W_hh, b_ih, b_hh):
    return run(
        dict(x=x, hidden=hidden, W_ih=W_ih, W_hh=W_hh, b_ih=b_ih, b_hh=b_hh)
    )


# revision 28
# speedup vs baseline: 370.6087x; 1.0401x over previous
"""Trainium2 Bass kernel for the bug-faithful CTRNN problem.

Semantics (hidden is never updated across time in the reference):
    out[t] = relu(x[t] @ W_ih^T + b_ih + hidden @ W_hh^T + b_hh)
    return out, out[-1]

This collapses to one large matmul over (T*B, I) @ (I, H) plus a per-batch
bias term shared across time.

Distribution: data-parallel over T across 8 NeuronCores (each core gets
T/8 = 32 timesteps = 4096 rows); W_ih / W_hh / hidden / biases replicated.

Device mapping per core:
  - lhsT (stationary) = x^T tiles [i=128, r=128], fp32r (fp22-truncated fp32,
    full PE rate at N>=256)
  - rhs  (moving)     = W_ih^T tiles [i=128, h=512], fp32r
  - psum [r=128, h=512] accumulated over 4 k-tiles
  - bias_full[b, h] = hidden @ W_hh^T + b_ih + b_hh computed once on device
    via an augmented contraction (two ones-rows pick up b_ih and b_hh)
  - epilogue: DVE add (psum + bias) -> fp16 SBUF, ACT relu, DMA to DRAM
  - host: upcast fp16 -> fp32, concat shards
"""

import sys

if "/opt/trn_rl_repo" not in sys.path:
    sys.path.insert(0, "/opt/trn_rl_repo")

import numpy as np

import concourse.bacc as bacc
import concourse.bass as bass  # noqa: F401
import concourse.mybir as mybir
from concourse.bass_utils import run_bass_kernel_spmd
from concourse.tile import TileContext

T, B, I, H = 256, 128, 512, 1024
NCORES = 8
TCORE = T // NCORES          # timesteps per core
R = TCORE * B                # rows per core (4096)
KI = I // 128                # k-tiles for the input matmul (4)
KH = H // 128 + 1            # k-tiles for the hidden matmul, incl. bias rows (9)
RCHUNK = 512                 # rows per x DMA chunk
NCHUNKS = R // RCHUNK        # 8
RTILES = RCHUNK // 128       # r-tiles per chunk (4)

OUT_DT = mybir.dt.float16    # device output dtype (upcast to fp32 on host)

LAST_EXEC_NS = None          # filled in by run() when tracing is enabled
LAST_RESULT = None

_NC_CACHE = {}


def _dedup_ldweights(nc):
    """Drop InstLdweights that reload the exact weights already in the PE
    array (same physical AP as the previous load, no waits/updates). The
    tile legalizer pairs every matmul with its own ldweights; consecutive
    matmuls sharing a stationary operand pay a redundant ~100-200 ns array
    load each, which this removes."""
    PE = mybir.EngineType.PE
    removed = 0
    for bb in nc.main_func.blocks:
        prev_sig = None
        to_remove = []
        for ins in bb.instructions:
            if getattr(ins, "engine", None) != PE:
                continue
            t = type(ins).__name__
            if t == "InstLdweights":
                sig = str(ins.ins[0])
                if sig == prev_sig and not ins.has_wait() and not ins.has_update():
                    to_remove.append(ins)
                else:
                    prev_sig = sig
            elif t in ("InstMatmult", "InstMatmultMx"):
                if getattr(ins, "is_transpose", False) or getattr(
                    ins, "ldweights", False
                ):
                    prev_sig = None  # these (re)load the array themselves
            else:
                prev_sig = None  # unknown PE instruction: be conservative
        for ins in to_remove:
            bb.instructions.remove(ins)
        removed += len(to_remove)
    return removed


def _build(reps=1, loop_n=None, variant="full"):
    f32r = mybir.dt.float32r
    f32 = mybir.dt.float32
    f16 = mybir.dt.float16
    relu = mybir.ActivationFunctionType.Relu

    nc = bacc.Bacc("TRN2", target_bir_lowering=False)
    xt_d = nc.declare_dram_parameter(
        "xt", [NCHUNKS, 128, KI, RCHUNK], f16, isOutput=False
    )
    wt_d = nc.declare_dram_parameter("wt", [128, KI, H], f16, isOutput=False)
    hid_d = nc.declare_dram_parameter("hid", [128, KH, B], f16, isOutput=False)
    whh_d = nc.declare_dram_parameter("whh", [128, KH, H], f16, isOutput=False)
    out_d = nc.declare_dram_parameter("out", [R // 128, 128, H], OUT_DT, isOutput=True)

    with TileContext(nc) as tc:
        wh_bufs = 2 if variant == "full" else 1
        with (
            tc.tile_pool(name="wpool", bufs=wh_bufs) as wpool,
            tc.tile_pool(name="hpool", bufs=wh_bufs) as hpool,
            tc.tile_pool(name="xpool", bufs=4) as xpool,
            tc.tile_pool(name="opool", bufs=6) as opool,
            tc.tile_pool(name="psum_main", bufs=6, space="PSUM") as psum_main,
            tc.tile_pool(name="psum_bias", bufs=2, space="PSUM") as psum_bias,
        ):

            def emit_body():
                # Input loads on the SP (sync) HWDGE ring.
                wt_sb = wpool.tile([128, KI, H], f16)
                nc.sync.dma_start(out=wt_sb[:], in_=wt_d[:])

                xc0 = xpool.tile([128, KI, RCHUNK], f16)
                nc.sync.dma_start(out=xc0[:], in_=xt_d[:][0])

                # Hidden-term operands on the ACT (scalar) HWDGE ring, which
                # is otherwise idle early (it carries output stores later).
                hid_sb = hpool.tile([128, KH, B], f16)
                nc.scalar.dma_start(out=hid_sb[:], in_=hid_d[:])
                whh_sb = hpool.tile([128, KH, H], f16)
                nc.scalar.dma_start(out=whh_sb[:], in_=whh_d[:])

                bias_sb = wpool.tile([128, H], f32)

                def emit_bias_stage():
                    for h2 in range(2):
                        ps = psum_bias.tile([128, 512], f32)
                        for kk in range(KH):
                            nc.tensor.matmul(
                                ps[:],
                                lhsT=hid_sb[:, kk, :],
                                rhs=whh_sb[:, kk, h2 * 512 : (h2 + 1) * 512],
                                start=(kk == 0),
                                stop=(kk == KH - 1),
                            )
                        nc.vector.tensor_copy(
                            out=bias_sb[:, h2 * 512 : (h2 + 1) * 512], in_=ps[:]
                        )

                def emit_rtile_mms(xc, rt):
                    # kk outer / h2 inner: the two matmuls of each kk share
                    # the same stationary xT tile, so after LDW-dedup the
                    # second one skips its weight reload.
                    ps_pair = [
                        psum_main.tile(
                            [128, 512], f32, name=f"psm_{rt}_{h2}", tag="psm"
                        )
                        for h2 in range(2)
                    ]
                    for kk in range(KI):
                        for h2 in range(2):
                            nc.tensor.matmul(
                                ps_pair[h2][:],
                                lhsT=xc[:, kk, rt * 128 : (rt + 1) * 128],
                                rhs=wt_sb[:, kk, h2 * 512 : (h2 + 1) * 512],
                                start=(kk == 0),
                                stop=(kk == KI - 1),
                            )
                    return ps_pair

                def emit_rtile_epilogue(ps_pair, m):
                    o_sb = opool.tile([128, H], OUT_DT)
                    for h2 in range(2):
                        nc.vector.tensor_add(
                            out=o_sb[:, h2 * 512 : (h2 + 1) * 512],
                            in0=ps_pair[h2][:],
                            in1=bias_sb[:, h2 * 512 : (h2 + 1) * 512],
                        )
                    nc.scalar.activation(out=o_sb[:], in_=o_sb[:], func=relu)
                    nc.scalar.dma_start(out=out_d[:][m], in_=o_sb[:])

                def emit_rtile(xc, m, rt):
                    emit_rtile_epilogue(emit_rtile_mms(xc, rt), m)

                # Chunk 0, r-tile 0 matmuls first so PE starts as soon as
                # wt+xc0 land; then the bias stage (its DVE copies must
                # precede r-tile 0's bias-consuming adds in the DVE stream,
                # and its matmuls must precede any matmul that transitively
                # waits on bias consumers -> no psum-slot deadlock); then
                # r-tile 0's epilogue.
                ps0 = emit_rtile_mms(xc0, 0)
                emit_bias_stage()
                emit_rtile_epilogue(ps0, 0)
                for rt in range(1, RTILES):
                    emit_rtile(xc0, rt, rt)
                for c in range(1, NCHUNKS):
                    xc = xpool.tile([128, KI, RCHUNK], f16)
                    nc.sync.dma_start(out=xc[:], in_=xt_d[:][c])
                    for rt in range(RTILES):
                        emit_rtile(xc, c * RTILES + rt, rt)

            def emit_variant_body(kind, x_tiles, wt_sb, hid_sb, whh_sb, o_sb):
                if kind == "mm":
                    for c in range(NCHUNKS):
                        for rt in range(RTILES):
                            for h2 in range(2):
                                ps = psum_main.tile([128, 512], f32)
                                for kk in range(KI):
                                    nc.tensor.matmul(
                                        ps[:],
                                        lhsT=x_tiles[c][:, kk, rt * 128 : (rt + 1) * 128],
                                        rhs=wt_sb[:, kk, h2 * 512 : (h2 + 1) * 512],
                                        start=(kk == 0),
                                        stop=(kk == KI - 1),
                                    )
                elif kind == "mm_shared":
                    for c in range(NCHUNKS):
                        for rt in range(RTILES):
                            for h2 in range(2):
                                ps = psum_main.tile([128, 512], f32)
                                for kk in range(KI):
                                    nc.tensor.matmul(
                                        ps[:],
                                        lhsT=x_tiles[0][:, 0, 0:128],
                                        rhs=wt_sb[:, 0, 0:512],
                                        start=(kk == 0),
                                        stop=(kk == KI - 1),
                                    )
                elif kind == "mm_sharedw":
                    # fixed stationary, varying moving operand
                    for c in range(NCHUNKS):
                        for rt in range(RTILES):
                            for h2 in range(2):
                                ps = psum_main.tile([128, 512], f32)
                                for kk in range(KI):
                                    nc.tensor.matmul(
                                        ps[:],
                                        lhsT=x_tiles[0][:, 0, 0:128],
                                        rhs=wt_sb[:, kk, h2 * 512 : (h2 + 1) * 512],
                                        start=(kk == 0),
                                        stop=(kk == KI - 1),
                                    )
                elif kind == "mm_reuse8":
                    # stationary = Wt tile reused across 8 consecutive MMs
                    # (transposed-output mapping)
                    for ht in range(H // 128):
                        pss = [
                            psum_main.tile(
                                [128, 512], f32, bufs=8, tag="ps8", name=f"ps8_{ht}_{i}"
                            )
                            for i in range(NCHUNKS)
                        ]
                        for kk in range(KI):
                            for rc in range(NCHUNKS):
                                nc.tensor.matmul(
                                    pss[rc][:],
                                    lhsT=wt_sb[:, kk, ht * 128 : (ht + 1) * 128],
                                    rhs=x_tiles[rc][:, kk, :],
                                    start=(kk == 0),
                                    stop=(kk == KI - 1),
                                )
                elif kind == "dma":
                    wt2 = wpool.tile([128, KI, H], f16, bufs=1, tag="wt2")
                    nc.sync.dma_start(out=wt2[:], in_=wt_d[:])
                    hid2 = hpool.tile([128, KH, B], f16, bufs=1, tag="hid2")
                    nc.scalar.dma_start(out=hid2[:], in_=hid_d[:])
                    whh2 = hpool.tile([128, KH, H], f16, bufs=1, tag="whh2")
                    nc.scalar.dma_start(out=whh2[:], in_=whh_d[:])
                    for c in range(NCHUNKS):
                        xc = xpool.tile([128, KI, RCHUNK], f16)
                        nc.sync.dma_start(out=xc[:], in_=xt_d[:][c])
                        for rt in range(RTILES):
                            nc.scalar.dma_start(
                                out=out_d[:][c * RTILES + rt], in_=o_sb[:]
                            )

            if variant != "full":
                # Bottleneck-bisection variants (timing only, wrong output).
                wt_sb = wpool.tile([128, KI, H], f16)
                nc.sync.dma_start(out=wt_sb[:], in_=wt_d[:])
                hid_sb = hpool.tile([128, KH, B], f16)
                nc.scalar.dma_start(out=hid_sb[:], in_=hid_d[:])
                whh_sb = hpool.tile([128, KH, H], f16)
                nc.scalar.dma_start(out=whh_sb[:], in_=whh_d[:])
                o_sb = opool.tile([128, H], OUT_DT)
                nc.any.memset(o_sb[:], 0.0)
                x_tiles = []
                if variant != "dma":
                    for c in range(NCHUNKS):
                        xc = xpool.tile(
                            [128, KI, RCHUNK], f16, bufs=NCHUNKS, tag="xres"
                        )
                        nc.sync.dma_start(out=xc[:], in_=xt_d[:][c])
                        x_tiles.append(xc)
                assert loop_n is not None
                with tc.For_i(0, loop_n, 1, hint_engines=(mybir.EngineType.PE,)):
                    emit_variant_body(variant, x_tiles, wt_sb, hid_sb, whh_sb, o_sb)
            elif loop_n is not None:
                # Timing-only variant: repeat the whole body in a hardware
                # loop so device exec time dominates host/transfer overhead.
                with tc.For_i(0, loop_n, 1, hint_engines=(mybir.EngineType.PE,)):
                    emit_body()
            else:
                for _ in range(reps):
                    emit_body()

    nc.compile()  # bacc pass pipeline: legalizes multi-wait instructions etc.
    _dedup_ldweights(nc)
    return nc


def _get_nc(reps=1, loop_n=None, variant="full"):
    key = (reps, loop_n, variant)
    if key not in _NC_CACHE:
        _NC_CACHE[key] = _build(reps, loop_n, variant)
    return _NC_CACHE[key]


def _prep_inputs(x, hidden, W_ih, W_hh, b_ih, b_hh):
    x = np.ascontiguousarray(np.asarray(x, dtype=np.float32))
    hidden = np.asarray(hidden, dtype=np.float32)
    W_ih = np.asarray(W_ih, dtype=np.float32)
    W_hh = np.asarray(W_hh, dtype=np.float32)
    b_ih = np.asarray(b_ih, dtype=np.float32)
    b_hh = np.asarray(b_hh, dtype=np.float32)

    # wt[p, k, h] = W_ih[h, k*128+p], fp16 moving operand
    wt = np.ascontiguousarray(
        W_ih.T.reshape(KI, 128, H).transpose(1, 0, 2).astype(np.float16)
    )

    # Augmented hidden^T: rows 0..H-1 = hidden.T, rows H and H+1 = 1.0 (they
    # pick up the b_ih and b_hh rows of the augmented W_hh^T), rest zero.
    ha = np.zeros((KH * 128, B), dtype=np.float32)
    ha[:H] = hidden.T
    ha[H] = 1.0
    ha[H + 1] = 1.0
    hid = np.ascontiguousarray(
        ha.reshape(KH, 128, B).transpose(1, 0, 2).astype(np.float16)
    )

    wa = np.zeros((KH * 128, H), dtype=np.float32)
    wa[:H] = W_hh.T
    wa[H] = b_ih
    wa[H + 1] = b_hh
    whh = np.ascontiguousarray(
        wa.reshape(KH, 128, H).transpose(1, 0, 2).astype(np.float16)
    )

    in_maps = []
    for c in range(NCORES):
        xs = x[c * TCORE : (c + 1) * TCORE].reshape(R, I)
        # xt[c8, p, k, r] = xs[c8*RCHUNK + r, k*128 + p]
        xt = np.ascontiguousarray(
            xs.reshape(NCHUNKS, RCHUNK, KI, 128).transpose(0, 3, 2, 1)
        ).astype(np.float16)
        in_maps.append({"xt": xt, "wt": wt, "hid": hid, "whh": whh})
    return in_maps


def run(inputs, trace=False, tmpdir=None, reps=1, loop_n=None, variant="full"):
    global LAST_EXEC_NS, LAST_RESULT
    nc = _get_nc(reps, loop_n, variant)
    in_maps = _prep_inputs(**inputs)
    res = run_bass_kernel_spmd(
        nc, in_maps, core_ids=list(range(NCORES)), trace=trace, tmpdir=tmpdir
    )
    LAST_RESULT = res
    LAST_EXEC_NS = res.exec_time_ns
    out = np.empty((T, B, H), dtype=np.float32)
    for c in range(NCORES):
        shard = res.results[c]["out"]  # (R//128, 128, H) fp16
        out[c * TCORE : (c + 1) * TCORE] = shard.astype(np.float32).reshape(
            TCORE, B, H
        )
    return out, out[-1]


def kernel(x, hidden, W_ih, W_hh, b_ih, b_hh):
    return run(
        dict(x=x, hidden=hidden, W_ih=W_ih, W_hh=W_hh, b_ih=b_ih, b_hh=b_hh)
    )


# revision 36
# speedup vs baseline: 382.9949x; 1.0334x over previous
"""Trainium2 Bass kernel for the bug-faithful CTRNN problem.

Semantics (hidden is never updated across time in the reference):
    out[t] = relu(x[t] @ W_ih^T + b_ih + hidden @ W_hh^T + b_hh)
    return out, out[-1]

This collapses to one large matmul over (T*B, I) @ (I, H) plus a per-batch
bias term shared across time.

Distribution: data-parallel over T across 8 NeuronCores (each core gets
T/8 = 32 timesteps = 4096 rows); W_ih / W_hh / hidden / biases replicated.

Device mapping per core:
  - lhsT (stationary) = x^T tiles [i=128, r=128], fp32r (fp22-truncated fp32,
    full PE rate at N>=256)
  - rhs  (moving)     = W_ih^T tiles [i=128, h=512], fp32r
  - psum [r=128, h=512] accumulated over 4 k-tiles
  - bias_full[b, h] = hidden @ W_hh^T + b_ih + b_hh computed once on device
    via an augmented contraction (two ones-rows pick up b_ih and b_hh)
  - epilogue: DVE add (psum + bias) -> fp16 SBUF, ACT relu, DMA to DRAM
  - host: upcast fp16 -> fp32, concat shards
"""

import sys

if "/opt/trn_rl_repo" not in sys.path:
    sys.path.insert(0, "/opt/trn_rl_repo")

import numpy as np

import concourse.bacc as bacc
import concourse.bass as bass  # noqa: F401
import concourse.mybir as mybir
from concourse.bass_utils import run_bass_kernel_spmd
from concourse.tile import TileContext

T, B, I, H = 256, 128, 512, 1024
NCORES = 8
TCORE = T // NCORES          # timesteps per core
R = TCORE * B                # rows per core (4096)
KI = I // 128                # k-tiles for the input matmul (4)
KH = H // 128 + 1            # k-tiles for the hidden matmul, incl. bias rows (9)
RCHUNK = 512                 # rows per x DMA chunk
NCHUNKS = R // RCHUNK        # 8
RTILES = RCHUNK // 128       # r-tiles per chunk (4)

OUT_DT = mybir.dt.float16    # device output dtype (upcast to fp32 on host)

LAST_EXEC_NS = None          # filled in by run() when tracing is enabled
LAST_RESULT = None

_NC_CACHE = {}


def _dedup_ldweights(nc):
    """Drop InstLdweights that reload the exact weights already in the PE
    array (same physical AP as the previous load, no waits/updates). The
    tile legalizer pairs every matmul with its own ldweights; consecutive
    matmuls sharing a stationary operand pay a redundant ~100-200 ns array
    load each, which this removes."""
    PE = mybir.EngineType.PE
    removed = 0
    for bb in nc.main_func.blocks:
        prev_sig = None
        to_remove = []
        for ins in bb.instructions:
            if getattr(ins, "engine", None) != PE:
                continue
            t = type(ins).__name__
            if t == "InstLdweights":
                sig = str(ins.ins[0])
                if sig == prev_sig and not ins.has_wait() and not ins.has_update():
                    to_remove.append(ins)
                else:
                    prev_sig = sig
            elif t in ("InstMatmult", "InstMatmultMx"):
                if getattr(ins, "is_transpose", False) or getattr(
                    ins, "ldweights", False
                ):
                    prev_sig = None  # these (re)load the array themselves
            else:
                prev_sig = None  # unknown PE instruction: be conservative
        for ins in to_remove:
            bb.instructions.remove(ins)
        removed += len(to_remove)
    return removed


def _build(reps=1, loop_n=None, variant="full"):
    f32r = mybir.dt.float32r
    f32 = mybir.dt.float32
    f16 = mybir.dt.float16
    relu = mybir.ActivationFunctionType.Relu

    nc = bacc.Bacc("TRN2", target_bir_lowering=False)
    xt_d = nc.declare_dram_parameter(
        "xt", [NCHUNKS, 128, KI, RCHUNK], f16, isOutput=False
    )
    wt_d = nc.declare_dram_parameter("wt", [128, KI, H], f16, isOutput=False)
    hid_d = nc.declare_dram_parameter("hid", [128, KH, B], f16, isOutput=False)
    whh_d = nc.declare_dram_parameter("whh", [128, KH, H], f16, isOutput=False)
    out_d = nc.declare_dram_parameter("out", [R // 128, 128, H], OUT_DT, isOutput=True)

    with TileContext(nc) as tc:
        wh_bufs = 2 if variant == "full" else 1
        with (
            tc.tile_pool(name="wpool", bufs=wh_bufs) as wpool,
            tc.tile_pool(name="hpool", bufs=wh_bufs) as hpool,
            tc.tile_pool(name="xpool", bufs=8) as xpool,
            tc.tile_pool(name="opool", bufs=8) as opool,
            tc.tile_pool(name="psum_main", bufs=6, space="PSUM") as psum_main,
            tc.tile_pool(name="psum_bias", bufs=2, space="PSUM") as psum_bias,
        ):

            def emit_body():
                # Input loads on the SP (sync) HWDGE ring.
                wt_sb = wpool.tile([128, KI, H], f16)
                nc.sync.dma_start(out=wt_sb[:], in_=wt_d[:])

                xc0 = xpool.tile([128, KI, RCHUNK], f16)
                nc.sync.dma_start(out=xc0[:], in_=xt_d[:][0])

                # Hidden-term operands on the ACT (scalar) HWDGE ring, which
                # is otherwise idle early (it carries output stores later).
                hid_sb = hpool.tile([128, KH, B], f16)
                nc.scalar.dma_start(out=hid_sb[:], in_=hid_d[:])
                whh_sb = hpool.tile([128, KH, H], f16)
                nc.scalar.dma_start(out=whh_sb[:], in_=whh_d[:])

                bias_sb = wpool.tile([128, H], f32)

                def emit_bias_stage():
                    for h2 in range(2):
                        ps = psum_bias.tile([128, 512], f32)
                        for kk in range(KH):
                            nc.tensor.matmul(
                                ps[:],
                                lhsT=hid_sb[:, kk, :],
                                rhs=whh_sb[:, kk, h2 * 512 : (h2 + 1) * 512],
                                start=(kk == 0),
                                stop=(kk == KH - 1),
                            )
                        nc.vector.tensor_copy(
                            out=bias_sb[:, h2 * 512 : (h2 + 1) * 512], in_=ps[:]
                        )

                def emit_rtile_mms(xc, rt):
                    # kk outer / h2 inner: the two matmuls of each kk share
                    # the same stationary xT tile, so after LDW-dedup the
                    # second one skips its weight reload.
                    ps_pair = [
                        psum_main.tile(
                            [128, 512], f32, name=f"psm_{rt}_{h2}", tag="psm"
                        )
                        for h2 in range(2)
                    ]
                    for kk in range(KI):
                        for h2 in range(2):
                            nc.tensor.matmul(
                                ps_pair[h2][:],
                                lhsT=xc[:, kk, rt * 128 : (rt + 1) * 128],
                                rhs=wt_sb[:, kk, h2 * 512 : (h2 + 1) * 512],
                                start=(kk == 0),
                                stop=(kk == KI - 1),
                            )
                    return ps_pair

                def emit_rtile_epilogue(ps_pair, m):
                    o_sb = opool.tile([128, H], OUT_DT)
                    for h2 in range(2):
                        nc.vector.tensor_add(
                            out=o_sb[:, h2 * 512 : (h2 + 1) * 512],
                            in0=ps_pair[h2][:],
                            in1=bias_sb[:, h2 * 512 : (h2 + 1) * 512],
                        )
                    nc.scalar.activation(out=o_sb[:], in_=o_sb[:], func=relu)
                    nc.scalar.dma_start(out=out_d[:][m], in_=o_sb[:])

                def emit_rtile(xc, m, rt):
                    emit_rtile_epilogue(emit_rtile_mms(xc, rt), m)

                # Chunk 0, r-tile 0 matmuls first so PE starts as soon as
                # wt+xc0 land; then the bias stage (its DVE copies must
                # precede r-tile 0's bias-consuming adds in the DVE stream,
                # and its matmuls must precede any matmul that transitively
                # waits on bias consumers -> no psum-slot deadlock); then
                # r-tile 0's epilogue.
                ps0 = emit_rtile_mms(xc0, 0)
                emit_bias_stage()
                emit_rtile_epilogue(ps0, 0)
                for rt in range(1, RTILES):
                    emit_rtile(xc0, rt, rt)
                for c in range(1, NCHUNKS):
                    xc = xpool.tile([128, KI, RCHUNK], f16)
                    nc.sync.dma_start(out=xc[:], in_=xt_d[:][c])
                    for rt in range(RTILES):
                        emit_rtile(xc, c * RTILES + rt, rt)

            def emit_variant_body(kind, x_tiles, wt_sb, hid_sb, whh_sb, o_sb):
                if kind in ("mm", "mm_bf16"):
                    for c in range(NCHUNKS):
                        for rt in range(RTILES):
                            for h2 in range(2):
                                ps = psum_main.tile([128, 512], f32)
                                for kk in range(KI):
                                    nc.tensor.matmul(
                                        ps[:],
                                        lhsT=x_tiles[c][:, kk, rt * 128 : (rt + 1) * 128],
                                        rhs=wt_sb[:, kk, h2 * 512 : (h2 + 1) * 512],
                                        start=(kk == 0),
                                        stop=(kk == KI - 1),
                                    )
                elif kind == "mm_shared":
                    for c in range(NCHUNKS):
                        for rt in range(RTILES):
                            for h2 in range(2):
                                ps = psum_main.tile([128, 512], f32)
                                for kk in range(KI):
                                    nc.tensor.matmul(
                                        ps[:],
                                        lhsT=x_tiles[0][:, 0, 0:128],
                                        rhs=wt_sb[:, 0, 0:512],
                                        start=(kk == 0),
                                        stop=(kk == KI - 1),
                                    )
                elif kind == "mm_rg2":
                    # split contraction into two concurrent row-group halves
                    for c in range(NCHUNKS):
                        for rt in range(RTILES):
                            for h2 in range(2):
                                psA = psum_main.tile(
                                    [128, 512], f32, tag="psm", bufs=8,
                                    name=f"psA_{c}_{rt}_{h2}",
                                )
                                psB = psum_main.tile(
                                    [128, 512], f32, tag="psm", bufs=8,
                                    name=f"psB_{c}_{rt}_{h2}",
                                )
                                for kk in range(KI):
                                    nc.tensor.matmul(
                                        psA[:],
                                        lhsT=x_tiles[c][0:64, kk, rt * 128 : (rt + 1) * 128],
                                        rhs=wt_sb[0:64, kk, h2 * 512 : (h2 + 1) * 512],
                                        start=(kk == 0),
                                        stop=(kk == KI - 1),
                                        tile_position=(0, 0),
                                    )
                                    nc.tensor.matmul(
                                        psB[:],
                                        lhsT=x_tiles[c][64:128, kk, rt * 128 : (rt + 1) * 128],
                                        rhs=wt_sb[64:128, kk, h2 * 512 : (h2 + 1) * 512],
                                        start=(kk == 0),
                                        stop=(kk == KI - 1),
                                        tile_position=(64, 0),
                                    )
                elif kind == "mm_n256":
                    # same FLOPs via 512 MMs of N=256
                    for c in range(NCHUNKS):
                        for rt in range(RTILES):
                            for h4 in range(4):
                                ps = psum_main.tile(
                                    [128, 256], f32, tag="psm",
                                    name=f"psn_{c}_{rt}_{h4}",
                                )
                                for kk in range(KI):
                                    nc.tensor.matmul(
                                        ps[:],
                                        lhsT=x_tiles[c][:, kk, rt * 128 : (rt + 1) * 128],
                                        rhs=wt_sb[:, kk, h4 * 256 : (h4 + 1) * 256],
                                        start=(kk == 0),
                                        stop=(kk == KI - 1),
                                    )
                elif kind == "mm_dedup":
                    # kk outer / h2 inner (same as full kernel main loop):
                    # adjacent matmul pairs share lhsT -> dedup halves LDWs
                    for c in range(NCHUNKS):
                        for rt in range(RTILES):
                            pp2 = [
                                psum_main.tile(
                                    [128, 512], f32, tag="psm",
                                    name=f"psmv_{c}_{rt}_{h2}",
                                )
                                for h2 in range(2)
                            ]
                            for kk in range(KI):
                                for h2 in range(2):
                                    nc.tensor.matmul(
                                        pp2[h2][:],
                                        lhsT=x_tiles[c][:, kk, rt * 128 : (rt + 1) * 128],
                                        rhs=wt_sb[:, kk, h2 * 512 : (h2 + 1) * 512],
                                        start=(kk == 0),
                                        stop=(kk == KI - 1),
                                    )
                elif kind == "mm_sharedw":
                    # fixed stationary, varying moving operand
                    for c in range(NCHUNKS):
                        for rt in range(RTILES):
                            for h2 in range(2):
                                ps = psum_main.tile([128, 512], f32)
                                for kk in range(KI):
                                    nc.tensor.matmul(
                                        ps[:],
                                        lhsT=x_tiles[0][:, 0, 0:128],
                                        rhs=wt_sb[:, kk, h2 * 512 : (h2 + 1) * 512],
                                        start=(kk == 0),
                                        stop=(kk == KI - 1),
                                    )
                elif kind == "mm_reuse8":
                    # stationary = Wt tile reused across 8 consecutive MMs
                    # (transposed-output mapping)
                    for ht in range(H // 128):
                        pss = [
                            psum_main.tile(
                                [128, 512], f32, bufs=8, tag="ps8", name=f"ps8_{ht}_{i}"
                            )
                            for i in range(NCHUNKS)
                        ]
                        for kk in range(KI):
                            for rc in range(NCHUNKS):
                                nc.tensor.matmul(
                                    pss[rc][:],
                                    lhsT=wt_sb[:, kk, ht * 128 : (ht + 1) * 128],
                                    rhs=x_tiles[rc][:, kk, :],
                                    start=(kk == 0),
                                    stop=(kk == KI - 1),
                                )
                elif kind == "dma":
                    wt2 = wpool.tile([128, KI, H], f16, bufs=1, tag="wt2")
                    nc.sync.dma_start(out=wt2[:], in_=wt_d[:])
                    hid2 = hpool.tile([128, KH, B], f16, bufs=1, tag="hid2")
                    nc.scalar.dma_start(out=hid2[:], in_=hid_d[:])
                    whh2 = hpool.tile([128, KH, H], f16, bufs=1, tag="whh2")
                    nc.scalar.dma_start(out=whh2[:], in_=whh_d[:])
                    for c in range(NCHUNKS):
                        xc = xpool.tile([128, KI, RCHUNK], f16)
                        nc.sync.dma_start(out=xc[:], in_=xt_d[:][c])
                        for rt in range(RTILES):
                            nc.scalar.dma_start(
                                out=out_d[:][c * RTILES + rt], in_=o_sb[:]
                            )

            if variant != "full":
                # Bottleneck-bisection variants (timing only, wrong output).
                wt_sb = wpool.tile([128, KI, H], f16)
                nc.sync.dma_start(out=wt_sb[:], in_=wt_d[:])
                hid_sb = hpool.tile([128, KH, B], f16)
                nc.scalar.dma_start(out=hid_sb[:], in_=hid_d[:])
                whh_sb = hpool.tile([128, KH, H], f16)
                nc.scalar.dma_start(out=whh_sb[:], in_=whh_d[:])
                o_sb = opool.tile([128, H], OUT_DT)
                nc.any.memset(o_sb[:], 0.0)
                x_tiles = []
                if variant != "dma":
                    for c in range(NCHUNKS):
                        xc = xpool.tile(
                            [128, KI, RCHUNK], f16, bufs=NCHUNKS, tag="xres"
                        )
                        nc.sync.dma_start(out=xc[:], in_=xt_d[:][c])
                        x_tiles.append(xc)
                if variant == "mm_bf16":
                    bf16 = mybir.dt.bfloat16
                    wt_bf = wpool.tile([128, KI, H], bf16, bufs=1, tag="wtbf")
                    nc.vector.tensor_copy(out=wt_bf[:], in_=wt_sb[:])
                    wt_sb = wt_bf
                    xb_tiles = []
                    for c in range(NCHUNKS):
                        xb = xpool.tile(
                            [128, KI, RCHUNK], bf16, bufs=NCHUNKS, tag="xbres",
                            name=f"xb_{c}",
                        )
                        nc.vector.tensor_copy(out=xb[:], in_=x_tiles[c][:])
                        xb_tiles.append(xb)
                    x_tiles = xb_tiles
                assert loop_n is not None
                with tc.For_i(0, loop_n, 1, hint_engines=(mybir.EngineType.PE,)):
                    emit_variant_body(variant, x_tiles, wt_sb, hid_sb, whh_sb, o_sb)
            elif loop_n is not None:
                # Timing-only variant: repeat the whole body in a hardware
                # loop so device exec time dominates host/transfer overhead.
                with tc.For_i(0, loop_n, 1, hint_engines=(mybir.EngineType.PE,)):
                    emit_body()
            else:
                for _ in range(reps):
                    emit_body()

    nc.compile()  # bacc pass pipeline: legalizes multi-wait instructions etc.
    _dedup_ldweights(nc)
    return nc


def _get_nc(reps=1, loop_n=None, variant="full"):
    key = (reps, loop_n, variant)
    if key not in _NC_CACHE:
        _NC_CACHE[key] = _build(reps, loop_n, variant)
    return _NC_CACHE[key]


def _prep_inputs(x, hidden, W_ih, W_hh, b_ih, b_hh):
    x = np.ascontiguousarray(np.asarray(x, dtype=np.float32))
    hidden = np.asarray(hidden, dtype=np.float32)
    W_ih = np.asarray(W_ih, dtype=np.float32)
    W_hh = np.asarray(W_hh, dtype=np.float32)
    b_ih = np.asarray(b_ih, dtype=np.float32)
    b_hh = np.asarray(b_hh, dtype=np.float32)

    # wt[p, k, h] = W_ih[h, k*128+p], fp16 moving operand
    wt = np.ascontiguousarray(
        W_ih.T.reshape(KI, 128, H).transpose(1, 0, 2).astype(np.float16)
    )

    # Augmented hidden^T: rows 0..H-1 = hidden.T, rows H and H+1 = 1.0 (they
    # pick up the b_ih and b_hh rows of the augmented W_hh^T), rest zero.
    ha = np.zeros((KH * 128, B), dtype=np.float32)
    ha[:H] = hidden.T
    ha[H] = 1.0
    ha[H + 1] = 1.0
    hid = np.ascontiguousarray(
        ha.reshape(KH, 128, B).transpose(1, 0, 2).astype(np.float16)
    )

    wa = np.zeros((KH * 128, H), dtype=np.float32)
    wa[:H] = W_hh.T
    wa[H] = b_ih
    wa[H + 1] = b_hh
    whh = np.ascontiguousarray(
        wa.reshape(KH, 128, H).transpose(1, 0, 2).astype(np.float16)
    )

    in_maps = []
    for c in range(NCORES):
        xs = x[c * TCORE : (c + 1) * TCORE].reshape(R, I)
        # xt[c8, p, k, r] = xs[c8*RCHUNK + r, k*128 + p]
        xt = np.ascontiguousarray(
            xs.reshape(NCHUNKS, RCHUNK, KI, 128).transpose(0, 3, 2, 1)
        ).astype(np.float16)
        in_maps.append({"xt": xt, "wt": wt, "hid": hid, "whh": whh})
    return in_maps


def prep(inputs):
    return _prep_inputs(**inputs)


def run(inputs, trace=False, tmpdir=None, reps=1, loop_n=None, variant="full",
        in_maps=None, ncores=NCORES):
    global LAST_EXEC_NS, LAST_RESULT
    nc = _get_nc(reps, loop_n, variant)
    if in_maps is None:
        in_maps = _prep_inputs(**inputs)
    res = run_bass_kernel_spmd(
        nc, in_maps[:ncores], core_ids=list(range(ncores)), trace=trace,
        tmpdir=tmpdir
    )
    LAST_RESULT = res
    LAST_EXEC_NS = res.exec_time_ns
    out = np.empty((T, B, H), dtype=np.float32)
    for c in range(ncores):
        shard = res.results[c]["out"]  # (R//128, 128, H) fp16
        out[c * TCORE : (c + 1) * TCORE] = shard.astype(np.float32).reshape(
            TCORE, B, H
        )
    return out, out[-1]


def kernel(x, hidden, W_ih, W_hh, b_ih, b_hh):
    return run(
        dict(x=x, hidden=hidden, W_ih=W_ih, W_hh=W_hh, b_ih=b_ih, b_hh=b_hh)
    )
